# revision 25
# baseline (speedup 1.0000x reference)
"""Causal self-attention (B=4, T=2048, d_model=1024, 16 heads) on 8 NeuronCores.

Sharding: core c = (batch b = c//2, head-group hg = c%2). Each core computes
QKV for its 8 heads, causal attention, and a partial output projection for its
batch. Host sums the two per-batch partials and adds biases (v-bias folds to a
constant through softmax: sum_k P = 1).
"""

import os
import sys

import numpy as np

for _p in ("/opt/trn_rl_repo", os.path.expanduser("~/.axon_site/_ro/trn_rl_repo")):
    if os.path.isdir(_p) and _p not in sys.path:
        sys.path.insert(0, _p)

D_MODEL = 1024
N_HEAD = 16
D_HEAD = 64
B = 4
T = 2048
HPC = 8          # heads per core
N_CORES = 8
NT = T // 128    # 16 T-tiles
NKT = D_MODEL // 128  # 8 contraction tiles
NPAIR = HPC // 2  # 4 head pairs per core
QCH = 512        # q-chunk width in attention
NCH = T // QCH   # 4 q-chunks

_CACHE = {}


def _build_nc():
    import concourse.tile as tile
    from concourse import bacc, mybir

    f32 = mybir.dt.float32
    f32r = mybir.dt.float32r
    Exp = mybir.ActivationFunctionType.Exp
    Ln = mybir.ActivationFunctionType.Ln

    nc = bacc.Bacc("TRN2")

    x_d = nc.dram_tensor("x", [T, D_MODEL], f32, kind="ExternalInput")
    wqk_d = nc.dram_tensor("wqk", [D_MODEL, 1024], f32r, kind="ExternalInput")
    wv_d = nc.dram_tensor("wv", [D_MODEL, 512], f32r, kind="ExternalInput")
    wp_d = nc.dram_tensor("wp", [512, D_MODEL], f32r, kind="ExternalInput")
    bqk_d = nc.dram_tensor("bqk", [128, 8], f32, kind="ExternalInput")
    ident_d = nc.dram_tensor("ident", [128, 128], f32, kind="ExternalInput")
    masks_d = nc.dram_tensor("masks", [4, 128, QCH], f32r, kind="ExternalInput")
    ones_d = nc.dram_tensor("ones", [128, NT, 2, 64], f32r, kind="ExternalInput")
    out_d = nc.dram_tensor("out", [T, D_MODEL], f32, kind="ExternalOutput")

    with tile.TileContext(nc) as tc:
        with (
            tc.tile_pool(name="persist", bufs=1) as persist,
            tc.tile_pool(name="dram", bufs=1, space="DRAM") as dram,
        ):
            ident_sb = persist.tile([128, 128], f32)
            nc.default_dma_engine.dma_start(ident_sb[:], ident_d[:])
            masks_sb = persist.tile([128, 4, QCH], f32r)
            nc.default_dma_engine.dma_start(
                masks_sb[:], masks_d[:].rearrange("m p c -> p m c")
            )
            bqk_sb = persist.tile([128, 8], f32)
            nc.default_dma_engine.dma_start(bqk_sb[:], bqk_d[:])
            vg = persist.tile([128, NT, 2, 128], f32r)  # [v_h | ones] stationaries
            nc.default_dma_engine.dma_start(vg[:, :, :, 64:128], ones_d[:])

            v_all = persist.tile([128, NT, 512], f32r)   # v natural, [T-tile part, kt, heads*64]
            ytn = persist.tile([128, NPAIR, T], f32r)    # normalized y^T stacked per pair
            qkT_dts = [
                dram.tile([2, 128, T], f32r, tag=f"qkb{p}", name=f"qkb{p}")
                for p in range(NPAIR)
            ]

            xT_cm = tc.tile_pool(name="xTp", bufs=1)
            xT_pool = xT_cm.__enter__()
            xT = xT_pool.tile([128, NKT, T], f32r)       # x transposed

            # ---- Phase 1: load x, transpose 128x128 blocks on TensorE ----
            with (
                tc.tile_pool(name="xin", bufs=6) as xin,
                tc.tile_pool(name="pst", bufs=4, space="PSUM") as pst,
            ):
                for tt in range(NT):
                    xt = xin.tile([128, D_MODEL], f32, tag="xt")
                    nc.default_dma_engine.dma_start(
                        xt[:], x_d[128 * tt : 128 * (tt + 1), :]
                    )
                    for kt in range(NKT):
                        ps = pst.tile([128, 128], f32, tag="ps")
                        nc.tensor.transpose(
                            ps[:], xt[:, 128 * kt : 128 * (kt + 1)], ident_sb[:]
                        )
                        nc.vector.tensor_copy(
                            xT[:, kt, 128 * tt : 128 * (tt + 1)], ps[:]
                        )

            # ---- Phase 2a: q^T/k^T = W_qk^T @ x^T (W stationary), to DRAM ----
            wqk_r = wqk_d[:].rearrange("(kt p) c -> p kt c", p=128)
            with (
                tc.tile_pool(name="wqk", bufs=2) as wpool,
                tc.tile_pool(name="psqk", bufs=2, space="PSUM") as psqk,
            ):
                for m in range(8):
                    wt = wpool.tile([128, NKT, 128], f32r, tag="w")
                    nc.default_dma_engine.dma_start(
                        wt[:], wqk_r[:, :, 128 * m : 128 * (m + 1)]
                    )
                    for nh in range(2):  # halves of T
                        ps = psqk.tile([128, 1024], f32, tag="qk")
                        for kt in range(NKT):
                            for ncx in range(2):
                                nc.tensor.matmul(
                                    ps[:, 512 * ncx : 512 * (ncx + 1)],
                                    (wt[:, kt, :]),
                                    (xT[
                                            :,
                                            kt,
                                            1024 * nh
                                            + 512 * ncx : 1024 * nh
                                            + 512 * (ncx + 1),
                                        ]
                                    ),
                                    start=(kt == 0),
                                    stop=(kt == NKT - 1),
                                )
                        sbt = wpool.tile([128, 1024], f32r, tag="qkout")
                        nc.vector.tensor_scalar_add(sbt[:], ps[:], bqk_sb[:, m : m + 1])
                        nc.default_dma_engine.dma_start(
                            qkT_dts[m // 2][m % 2, :, 1024 * nh : 1024 * (nh + 1)],
                            sbt[:],
                        )

            # ---- Phase 2b: v = x @ W_v (x^T stationary), stays in SBUF ----
            wv_r = wv_d[:].rearrange("(kt p) c -> p kt c", p=128)
            with (
                tc.tile_pool(name="wv", bufs=1) as wvpool,
                tc.tile_pool(name="psv", bufs=2, space="PSUM") as psv,
            ):
                wv_sb = wvpool.tile([128, NKT, 512], f32r)
                nc.default_dma_engine.dma_start(wv_sb[:], wv_r)
                for tt in range(NT):
                    ps = psv.tile([128, 512], f32, tag="v")
                    for kt in range(NKT):
                        nc.tensor.matmul(
                            ps[:],
                            (xT[:, kt, 128 * tt : 128 * (tt + 1)]),
                            (wv_sb[:, kt, :]),
                            start=(kt == 0),
                            stop=(kt == NKT - 1),
                        )
                    nc.vector.tensor_copy(v_all[:, tt, :], ps[:])

            xT_cm.__exit__(None, None, None)

            # ---- Phase 3: attention per head pair ----
            with (
                tc.tile_pool(name="qkp", bufs=2) as qkpool,
                tc.tile_pool(name="pt", bufs=6) as ptpool,
                tc.tile_pool(name="nrm", bufs=4) as nrmpool,
                tc.tile_pool(name="psS", bufs=3, space="PSUM") as psS,
                tc.tile_pool(name="psY", bufs=2, space="PSUM") as psY,
            ):
                for pair in range(NPAIR):
                    qk = qkpool.tile([128, 2, T], f32r, tag="qkp")
                    nc.default_dma_engine.dma_start(
                        qk[:],
                        qkT_dts[pair][:].rearrange("a p t -> p a t"),
                    )
                    for h01 in range(2):
                        h = 2 * pair + h01
                        nc.vector.tensor_copy(
                            vg[:, :, h01, 0:64], v_all[:, :, 64 * h : 64 * (h + 1)]
                        )
                    for j in range(NCH):
                        nk = 4 * (j + 1)  # causal k-tiles for this q-chunk
                        for h01 in range(2):
                            base = 64 * h01
                            psy = psY.tile([128, QCH], f32, tag="y")
                            for g in range(nk // 2):
                                pss = psS.tile([128, 2, QCH], f32, tag="s")
                                for kkk in range(2):
                                    kk = 2 * g + kkk
                                    nc.tensor.matmul(
                                        pss[:, kkk, :],
                                        (qk[
                                                base : base + 64,
                                                1,
                                                128 * kk : 128 * (kk + 1),
                                            ]
                                        ),
                                        (qk[
                                                base : base + 64,
                                                0,
                                                QCH * j : QCH * (j + 1),
                                            ]
                                        ),
                                        start=True,
                                        stop=True,
                                    )
                                pt = ptpool.tile([128, 2, QCH], f32r, tag="pt")
                                nc.scalar.activation(
                                    pt[:],
                                    pss[:],
                                    func=Exp,
                                    scale=0.125,
                                )
                                # causal masking on the diagonal k-tiles
                                for kkk in range(2):
                                    kk = 2 * g + kkk
                                    if kk >= 4 * j:
                                        nc.gpsimd.tensor_mul(
                                            pt[:, kkk, :],
                                            pt[:, kkk, :],
                                            masks_sb[:, kk - 4 * j, :],
                                        )
                                for kkk in range(2):
                                    kk = 2 * g + kkk
                                    # stationary [v_h | ones]: rows 0:64 = y,
                                    # rows 64:128 = softmax sums (replicated)
                                    nc.tensor.matmul(
                                        psy[:],
                                        (vg[:, kk, h01, :]),
                                        (pt[:, kkk, :]),
                                        start=(kk == 0),
                                        stop=(kk == nk - 1),
                                    )
                            lns = nrmpool.tile([64, QCH], f32, tag="lns")
                            nc.scalar.activation(
                                lns[:], psy[64:128, :], func=Ln
                            )
                            rc = nrmpool.tile([64, QCH], f32, tag="rc")
                            nc.scalar.activation(
                                rc[:], lns[:], func=Exp, scale=-1.0
                            )
                            nc.vector.tensor_mul(
                                ytn[base : base + 64, pair, QCH * j : QCH * (j + 1)],
                                psy[0:64, :],
                                rc[:],
                            )

            # ---- Phase 4: partial projection out = y^T.T @ W_proj_rows ----
            wp_r = wp_d[:].rearrange("(pr p) c -> p pr c", p=128)
            with (
                tc.tile_pool(name="wp", bufs=1) as wppool,
                tc.tile_pool(name="outp", bufs=3) as outpool,
                tc.tile_pool(name="pspj", bufs=2, space="PSUM") as pspj,
            ):
                wp_sb = wppool.tile([128, NPAIR, D_MODEL], f32r)
                nc.default_dma_engine.dma_start(wp_sb[:], wp_r)
                for tt in range(NT):
                    ob = outpool.tile([128, D_MODEL], f32, tag="ob")
                    for ncx in range(2):
                        ps = pspj.tile([128, 512], f32, tag="pj")
                        for pr in range(NPAIR):
                            nc.tensor.matmul(
                                ps[:],
                                (ytn[:, pr, 128 * tt : 128 * (tt + 1)]),
                                (wp_sb[:, pr, 512 * ncx : 512 * (ncx + 1)]),
                                start=(pr == 0),
                                stop=(pr == NPAIR - 1),
                            )
                        nc.vector.tensor_copy(ob[:, 512 * ncx : 512 * (ncx + 1)], ps[:])
                    nc.default_dma_engine.dma_start(
                        out_d[128 * tt : 128 * (tt + 1), :], ob[:]
                    )

    nc.finalize()
    return nc


def get_nc():
    if "nc" not in _CACHE:
        _CACHE["nc"] = _build_nc()
    return _CACHE["nc"]


def make_host_constants():
    ident = np.eye(128, dtype=np.float32)
    # mask m: k = QCH*j + 128*m + p vs q = QCH*j + c -> valid 128*m + p <= c
    p = np.arange(128)[:, None]
    c = np.arange(QCH)[None, :]
    masks = np.stack(
        [(128 * m + p <= c).astype(np.float32) for m in range(4)]
    )
    return ident, masks


def make_in_maps(x, W_attn, b_attn, W_proj):
    x = np.ascontiguousarray(np.asarray(x, dtype=np.float32))
    W_attn = np.asarray(W_attn, dtype=np.float32)
    b_attn = np.asarray(b_attn, dtype=np.float32)
    W_proj = np.asarray(W_proj, dtype=np.float32)
    ident, masks = make_host_constants()
    in_maps = []
    for c in range(N_CORES):
        b, hg = c // 2, c % 2
        h0 = HPC * hg
        # column order per pair: [q_even(64) | q_odd(64)] then [k_even | k_odd]
        qcols, kcols, bcols = [], [], []
        for pr in range(NPAIR):
            he, ho = h0 + 2 * pr, h0 + 2 * pr + 1
            qc = list(range(64 * he, 64 * he + 64)) + list(range(64 * ho, 64 * ho + 64))
            kc = [D_MODEL + i for i in qc]
            qcols.append(qc)
            kcols.append(kc)
        cols = []
        for pr in range(NPAIR):
            cols += qcols[pr] + kcols[pr]
        wqk = np.ascontiguousarray(W_attn[:, cols])
        bqk = np.ascontiguousarray(b_attn[cols].reshape(8, 128).T)
        vcols = list(range(2 * D_MODEL + 64 * h0, 2 * D_MODEL + 64 * (h0 + HPC)))
        wv = np.ascontiguousarray(W_attn[:, vcols])
        wp = np.ascontiguousarray(W_proj[64 * h0 : 64 * (h0 + HPC), :])
        in_maps.append(
            {
                "x": x[b],
                "wqk": wqk,
                "wv": wv,
                "wp": wp,
                "bqk": bqk,
                "ident": ident,
                "masks": masks,
                "ones": np.ones((128, NT, 2, 64), dtype=np.float32),
            }
        )
    return in_maps


def kernel(x, W_attn, b_attn, W_proj, b_proj, **run_kwargs):
    from concourse.bass_utils import run_bass_kernel_spmd

    nc = get_nc()
    in_maps = make_in_maps(x, W_attn, b_attn, W_proj)
    res = run_bass_kernel_spmd(nc, in_maps, list(range(N_CORES)), **run_kwargs)
    _CACHE["last_results"] = res

    b_attn = np.asarray(b_attn, dtype=np.float32)
    W_proj = np.asarray(W_proj, dtype=np.float32)
    b_proj = np.asarray(b_proj, dtype=np.float32)
    bv = b_attn[2 * D_MODEL : 3 * D_MODEL]
    const = (bv @ W_proj + b_proj).astype(np.float32)
    out = np.empty((B, T, D_MODEL), dtype=np.float32)
    for b in range(B):
        out[b] = res.results[2 * b]["out"] + res.results[2 * b + 1]["out"] + const
    return out


# revision 26
# speedup vs baseline: 1.0328x; 1.0328x over previous
"""Causal self-attention (B=4, T=2048, d_model=1024, 16 heads) on 8 NeuronCores.

Sharding: core c = (batch b = c//2, head-group hg = c%2). Each core computes
QKV for its 8 heads, causal attention, and a partial output projection for its
batch. Host sums the two per-batch partials and adds biases (v-bias folds to a
constant through softmax: sum_k P = 1).
"""

import os
import sys

import numpy as np

for _p in ("/opt/trn_rl_repo", os.path.expanduser("~/.axon_site/_ro/trn_rl_repo")):
    if os.path.isdir(_p) and _p not in sys.path:
        sys.path.insert(0, _p)

D_MODEL = 1024
N_HEAD = 16
D_HEAD = 64
B = 4
T = 2048
HPC = 8          # heads per core
N_CORES = 8
NT = T // 128    # 16 T-tiles
NKT = D_MODEL // 128  # 8 contraction tiles
NPAIR = HPC // 2  # 4 head pairs per core
QCH = 512        # q-chunk width in attention
NCH = T // QCH   # 4 q-chunks

_CACHE = {}


def _build_nc():
    import concourse.tile as tile
    from concourse import bacc, mybir

    f32 = mybir.dt.float32
    f32r = mybir.dt.float32r
    Exp = mybir.ActivationFunctionType.Exp
    Ln = mybir.ActivationFunctionType.Ln

    nc = bacc.Bacc("TRN2")

    x_d = nc.dram_tensor("x", [T, D_MODEL], f32, kind="ExternalInput")
    wqk_d = nc.dram_tensor("wqk", [D_MODEL, 1024], f32r, kind="ExternalInput")
    wv_d = nc.dram_tensor("wv", [D_MODEL, 512], f32r, kind="ExternalInput")
    wp_d = nc.dram_tensor("wp", [512, D_MODEL], f32r, kind="ExternalInput")
    bqk_d = nc.dram_tensor("bqk", [128, 8], f32, kind="ExternalInput")
    ident_d = nc.dram_tensor("ident", [128, 128], f32, kind="ExternalInput")
    masks_d = nc.dram_tensor("masks", [4, 128, QCH], f32r, kind="ExternalInput")
    ones_d = nc.dram_tensor("ones", [128, NT, 2, 64], f32r, kind="ExternalInput")
    out_d = nc.dram_tensor("out", [T, D_MODEL], f32, kind="ExternalOutput")

    with tile.TileContext(nc) as tc:
        with (
            tc.tile_pool(name="persist", bufs=1) as persist,
            tc.tile_pool(name="dram", bufs=1, space="DRAM") as dram,
        ):
            ident_sb = persist.tile([128, 128], f32)
            nc.default_dma_engine.dma_start(ident_sb[:], ident_d[:])
            masks_sb = persist.tile([128, 4, QCH], f32r)
            nc.default_dma_engine.dma_start(
                masks_sb[:], masks_d[:].rearrange("m p c -> p m c")
            )
            bqk_sb = persist.tile([128, 8], f32)
            nc.default_dma_engine.dma_start(bqk_sb[:], bqk_d[:])
            vg = persist.tile([128, NT, 2, 128], f32r)  # [v_h | ones] stationaries
            nc.default_dma_engine.dma_start(vg[:, :, :, 64:128], ones_d[:])

            v_all = persist.tile([128, NT, 512], f32r)   # v natural, [T-tile part, kt, heads*64]
            ytn = persist.tile([128, NPAIR, T], f32r)    # normalized y^T stacked per pair
            qkT_dts = [
                dram.tile([2, 128, T], f32r, tag=f"qkb{p}", name=f"qkb{p}")
                for p in range(NPAIR)
            ]

            xT_cm = tc.tile_pool(name="xTp", bufs=1)
            xT_pool = xT_cm.__enter__()
            xT = xT_pool.tile([128, NKT, T], f32r)       # x transposed

            # ---- Phase 1: load x, transpose 128x128 blocks on TensorE ----
            with (
                tc.tile_pool(name="xin", bufs=6) as xin,
                tc.tile_pool(name="pst", bufs=4, space="PSUM") as pst,
            ):
                for tt in range(NT):
                    xt = xin.tile([128, D_MODEL], f32, tag="xt")
                    nc.default_dma_engine.dma_start(
                        xt[:], x_d[128 * tt : 128 * (tt + 1), :]
                    )
                    for kt in range(NKT):
                        ps = pst.tile([128, 128], f32, tag="ps")
                        nc.tensor.transpose(
                            ps[:], xt[:, 128 * kt : 128 * (kt + 1)], ident_sb[:]
                        )
                        nc.vector.tensor_copy(
                            xT[:, kt, 128 * tt : 128 * (tt + 1)], ps[:]
                        )

            # ---- Phase 2a: q^T/k^T = W_qk^T @ x^T (W stationary), to DRAM ----
            wqk_r = wqk_d[:].rearrange("(kt p) c -> p kt c", p=128)
            with (
                tc.tile_pool(name="wqk", bufs=2) as wpool,
                tc.tile_pool(name="psqk", bufs=2, space="PSUM") as psqk,
            ):
                for m in range(8):
                    wt = wpool.tile([128, NKT, 128], f32r, tag="w")
                    nc.default_dma_engine.dma_start(
                        wt[:], wqk_r[:, :, 128 * m : 128 * (m + 1)]
                    )
                    for nh in range(2):  # halves of T
                        ps = psqk.tile([128, 1024], f32, tag="qk")
                        for kt in range(NKT):
                            for ncx in range(2):
                                nc.tensor.matmul(
                                    ps[:, 512 * ncx : 512 * (ncx + 1)],
                                    (wt[:, kt, :]),
                                    (xT[
                                            :,
                                            kt,
                                            1024 * nh
                                            + 512 * ncx : 1024 * nh
                                            + 512 * (ncx + 1),
                                        ]
                                    ),
                                    start=(kt == 0),
                                    stop=(kt == NKT - 1),
                                )
                        sbt = wpool.tile([128, 1024], f32r, tag="qkout")
                        nc.vector.tensor_scalar_add(sbt[:], ps[:], bqk_sb[:, m : m + 1])
                        nc.default_dma_engine.dma_start(
                            qkT_dts[m // 2][m % 2, :, 1024 * nh : 1024 * (nh + 1)],
                            sbt[:],
                        )

            # ---- Phase 2b: v = x @ W_v (x^T stationary), stays in SBUF ----
            wv_r = wv_d[:].rearrange("(kt p) c -> p kt c", p=128)
            with (
                tc.tile_pool(name="wv", bufs=1) as wvpool,
                tc.tile_pool(name="psv", bufs=2, space="PSUM") as psv,
            ):
                wv_sb = wvpool.tile([128, NKT, 512], f32r)
                nc.default_dma_engine.dma_start(wv_sb[:], wv_r)
                for tt in range(NT):
                    ps = psv.tile([128, 512], f32, tag="v")
                    for kt in range(NKT):
                        nc.tensor.matmul(
                            ps[:],
                            (xT[:, kt, 128 * tt : 128 * (tt + 1)]),
                            (wv_sb[:, kt, :]),
                            start=(kt == 0),
                            stop=(kt == NKT - 1),
                        )
                    nc.vector.tensor_copy(v_all[:, tt, :], ps[:])

            xT_cm.__exit__(None, None, None)

            # ---- Phase 3: attention per head pair ----
            with (
                tc.tile_pool(name="qkp", bufs=2) as qkpool,
                tc.tile_pool(name="pt", bufs=6) as ptpool,
                tc.tile_pool(name="nrm", bufs=4) as nrmpool,
                tc.tile_pool(name="psS", bufs=3, space="PSUM") as psS,
                tc.tile_pool(name="psY", bufs=2, space="PSUM") as psY,
            ):
                for pair in range(NPAIR):
                    qk = qkpool.tile([128, 2, T], f32r, tag="qkp")
                    nc.default_dma_engine.dma_start(
                        qk[:],
                        qkT_dts[pair][:].rearrange("a p t -> p a t"),
                    )
                    for h01 in range(2):
                        h = 2 * pair + h01
                        nc.vector.tensor_copy(
                            vg[:, :, h01, 0:64], v_all[:, :, 64 * h : 64 * (h + 1)]
                        )
                    for j in range(NCH):
                        nk = 4 * (j + 1)  # causal k-tiles for this q-chunk
                        for h01 in range(2):
                            base = 64 * h01
                            psy = psY.tile([128, QCH], f32, tag="y")
                            for g in range(nk // 2):
                                pss = psS.tile([128, 2, QCH], f32, tag="s")
                                for kkk in range(2):
                                    kk = 2 * g + kkk
                                    nc.tensor.matmul(
                                        pss[:, kkk, :],
                                        (qk[
                                                base : base + 64,
                                                1,
                                                128 * kk : 128 * (kk + 1),
                                            ]
                                        ),
                                        (qk[
                                                base : base + 64,
                                                0,
                                                QCH * j : QCH * (j + 1),
                                            ]
                                        ),
                                        start=True,
                                        stop=True,
                                    )
                                pt = ptpool.tile([128, 2, QCH], f32r, tag="pt")
                                nc.scalar.activation(
                                    pt[:],
                                    pss[:],
                                    func=Exp,
                                    scale=0.125,
                                )
                                # causal masking on the diagonal k-tiles
                                for kkk in range(2):
                                    kk = 2 * g + kkk
                                    if kk >= 4 * j:
                                        nc.vector.tensor_mul(
                                            pt[:, kkk, :],
                                            pt[:, kkk, :],
                                            masks_sb[:, kk - 4 * j, :],
                                        )
                                for kkk in range(2):
                                    kk = 2 * g + kkk
                                    # stationary [v_h | ones]: rows 0:64 = y,
                                    # rows 64:128 = softmax sums (replicated)
                                    nc.tensor.matmul(
                                        psy[:],
                                        (vg[:, kk, h01, :]),
                                        (pt[:, kkk, :]),
                                        start=(kk == 0),
                                        stop=(kk == nk - 1),
                                    )
                            lns = nrmpool.tile([64, QCH], f32, tag="lns")
                            nc.scalar.activation(
                                lns[:], psy[64:128, :], func=Ln
                            )
                            rc = nrmpool.tile([64, QCH], f32, tag="rc")
                            nc.scalar.activation(
                                rc[:], lns[:], func=Exp, scale=-1.0
                            )
                            nc.vector.tensor_mul(
                                ytn[base : base + 64, pair, QCH * j : QCH * (j + 1)],
                                psy[0:64, :],
                                rc[:],
                            )

            # ---- Phase 4: partial projection out = y^T.T @ W_proj_rows ----
            wp_r = wp_d[:].rearrange("(pr p) c -> p pr c", p=128)
            with (
                tc.tile_pool(name="wp", bufs=1) as wppool,
                tc.tile_pool(name="outp", bufs=3) as outpool,
                tc.tile_pool(name="pspj", bufs=2, space="PSUM") as pspj,
            ):
                wp_sb = wppool.tile([128, NPAIR, D_MODEL], f32r)
                nc.default_dma_engine.dma_start(wp_sb[:], wp_r)
                for tt in range(NT):
                    ob = outpool.tile([128, D_MODEL], f32, tag="ob")
                    for ncx in range(2):
                        ps = pspj.tile([128, 512], f32, tag="pj")
                        for pr in range(NPAIR):
                            nc.tensor.matmul(
                                ps[:],
                                (ytn[:, pr, 128 * tt : 128 * (tt + 1)]),
                                (wp_sb[:, pr, 512 * ncx : 512 * (ncx + 1)]),
                                start=(pr == 0),
                                stop=(pr == NPAIR - 1),
                            )
                        nc.vector.tensor_copy(ob[:, 512 * ncx : 512 * (ncx + 1)], ps[:])
                    nc.default_dma_engine.dma_start(
                        out_d[128 * tt : 128 * (tt + 1), :], ob[:]
                    )

    nc.finalize()
    return nc


def get_nc():
    if "nc" not in _CACHE:
        _CACHE["nc"] = _build_nc()
    return _CACHE["nc"]


def make_host_constants():
    ident = np.eye(128, dtype=np.float32)
    # mask m: k = QCH*j + 128*m + p vs q = QCH*j + c -> valid 128*m + p <= c
    p = np.arange(128)[:, None]
    c = np.arange(QCH)[None, :]
    masks = np.stack(
        [(128 * m + p <= c).astype(np.float32) for m in range(4)]
    )
    return ident, masks


def make_in_maps(x, W_attn, b_attn, W_proj):
    x = np.ascontiguousarray(np.asarray(x, dtype=np.float32))
    W_attn = np.asarray(W_attn, dtype=np.float32)
    b_attn = np.asarray(b_attn, dtype=np.float32)
    W_proj = np.asarray(W_proj, dtype=np.float32)
    ident, masks = make_host_constants()
    in_maps = []
    for c in range(N_CORES):
        b, hg = c // 2, c % 2
        h0 = HPC * hg
        # column order per pair: [q_even(64) | q_odd(64)] then [k_even | k_odd]
        qcols, kcols, bcols = [], [], []
        for pr in range(NPAIR):
            he, ho = h0 + 2 * pr, h0 + 2 * pr + 1
            qc = list(range(64 * he, 64 * he + 64)) + list(range(64 * ho, 64 * ho + 64))
            kc = [D_MODEL + i for i in qc]
            qcols.append(qc)
            kcols.append(kc)
        cols = []
        for pr in range(NPAIR):
            cols += qcols[pr] + kcols[pr]
        wqk = np.ascontiguousarray(W_attn[:, cols])
        bqk = np.ascontiguousarray(b_attn[cols].reshape(8, 128).T)
        vcols = list(range(2 * D_MODEL + 64 * h0, 2 * D_MODEL + 64 * (h0 + HPC)))
        wv = np.ascontiguousarray(W_attn[:, vcols])
        wp = np.ascontiguousarray(W_proj[64 * h0 : 64 * (h0 + HPC), :])
        in_maps.append(
            {
                "x": x[b],
                "wqk": wqk,
                "wv": wv,
                "wp": wp,
                "bqk": bqk,
                "ident": ident,
                "masks": masks,
                "ones": np.ones((128, NT, 2, 64), dtype=np.float32),
            }
        )
    return in_maps


def kernel(x, W_attn, b_attn, W_proj, b_proj, **run_kwargs):
    from concourse.bass_utils import run_bass_kernel_spmd

    nc = get_nc()
    in_maps = make_in_maps(x, W_attn, b_attn, W_proj)
    res = run_bass_kernel_spmd(nc, in_maps, list(range(N_CORES)), **run_kwargs)
    _CACHE["last_results"] = res

    b_attn = np.asarray(b_attn, dtype=np.float32)
    W_proj = np.asarray(W_proj, dtype=np.float32)
    b_proj = np.asarray(b_proj, dtype=np.float32)
    bv = b_attn[2 * D_MODEL : 3 * D_MODEL]
    const = (bv @ W_proj + b_proj).astype(np.float32)
    out = np.empty((B, T, D_MODEL), dtype=np.float32)
    for b in range(B):
        out[b] = res.results[2 * b]["out"] + res.results[2 * b + 1]["out"] + const
    return out


# revision 27
# speedup vs baseline: 1.1209x; 1.0853x over previous
"""Causal self-attention (B=4, T=2048, d_model=1024, 16 heads) on 8 NeuronCores.

Sharding: core c = (batch b = c//2, head-group hg = c%2). Each core computes
QKV for its 8 heads, causal attention, and a partial output projection for its
batch. Host sums the two per-batch partials and adds biases (v-bias folds to a
constant through softmax: sum_k P = 1).
"""

import os
import sys

import numpy as np

for _p in ("/opt/trn_rl_repo", os.path.expanduser("~/.axon_site/_ro/trn_rl_repo")):
    if os.path.isdir(_p) and _p not in sys.path:
        sys.path.insert(0, _p)

D_MODEL = 1024
N_HEAD = 16
D_HEAD = 64
B = 4
T = 2048
HPC = 8          # heads per core
N_CORES = 8
NT = T // 128    # 16 T-tiles
NKT = D_MODEL // 128  # 8 contraction tiles
NPAIR = HPC // 2  # 4 head pairs per core
QCH = 512        # q-chunk width in attention
NCH = T // QCH   # 4 q-chunks

_CACHE = {}


def _build_nc():
    import concourse.tile as tile
    from concourse import bacc, mybir

    f32 = mybir.dt.float32
    f32r = mybir.dt.float32r
    bf16 = mybir.dt.bfloat16
    Exp = mybir.ActivationFunctionType.Exp
    Ln = mybir.ActivationFunctionType.Ln

    nc = bacc.Bacc("TRN2")

    x_d = nc.dram_tensor("x", [T, D_MODEL], f32, kind="ExternalInput")
    wqk_d = nc.dram_tensor("wqk", [D_MODEL, 1024], f32r, kind="ExternalInput")
    wv_d = nc.dram_tensor("wv", [D_MODEL, 512], f32r, kind="ExternalInput")
    wp_d = nc.dram_tensor("wp", [512, D_MODEL], f32r, kind="ExternalInput")
    bqk_d = nc.dram_tensor("bqk", [128, 8], f32, kind="ExternalInput")
    ident_d = nc.dram_tensor("ident", [128, 128], f32, kind="ExternalInput")
    masks_d = nc.dram_tensor("masks", [4, 128, QCH], bf16, kind="ExternalInput")
    ones_d = nc.dram_tensor("ones", [128, NT, 2, 64], bf16, kind="ExternalInput")
    out_d = nc.dram_tensor("out", [T, D_MODEL], f32, kind="ExternalOutput")

    with tile.TileContext(nc) as tc:
        with (
            tc.tile_pool(name="persist", bufs=1) as persist,
            tc.tile_pool(name="dram", bufs=1, space="DRAM") as dram,
        ):
            ident_sb = persist.tile([128, 128], f32)
            nc.default_dma_engine.dma_start(ident_sb[:], ident_d[:])
            masks_sb = persist.tile([128, 4, QCH], bf16)
            nc.default_dma_engine.dma_start(
                masks_sb[:], masks_d[:].rearrange("m p c -> p m c")
            )
            bqk_sb = persist.tile([128, 8], f32)
            nc.default_dma_engine.dma_start(bqk_sb[:], bqk_d[:])
            vg = persist.tile([128, NT, 2, 128], bf16)  # [v_h | ones] stationaries
            nc.default_dma_engine.dma_start(vg[:, :, :, 64:128], ones_d[:])

            v_all = persist.tile([128, NT, 512], bf16)   # v natural, [T-tile part, kt, heads*64]
            ytn = persist.tile([128, NPAIR, T], f32r)    # normalized y^T stacked per pair
            qkT_dts = [
                dram.tile([2, 128, T], bf16, tag=f"qkb{p}", name=f"qkb{p}")
                for p in range(NPAIR)
            ]

            xT_cm = tc.tile_pool(name="xTp", bufs=1)
            xT_pool = xT_cm.__enter__()
            xT = xT_pool.tile([128, NKT, T], f32r)       # x transposed

            # ---- Phase 1: load x, transpose 128x128 blocks on TensorE ----
            with (
                tc.tile_pool(name="xin", bufs=6) as xin,
                tc.tile_pool(name="pst", bufs=4, space="PSUM") as pst,
            ):
                for tt in range(NT):
                    xt = xin.tile([128, D_MODEL], f32, tag="xt")
                    nc.default_dma_engine.dma_start(
                        xt[:], x_d[128 * tt : 128 * (tt + 1), :]
                    )
                    for kt in range(NKT):
                        ps = pst.tile([128, 128], f32, tag="ps")
                        nc.tensor.transpose(
                            ps[:], xt[:, 128 * kt : 128 * (kt + 1)], ident_sb[:]
                        )
                        nc.vector.tensor_copy(
                            xT[:, kt, 128 * tt : 128 * (tt + 1)], ps[:]
                        )

            # ---- Phase 2a: q^T/k^T = W_qk^T @ x^T (W stationary), to DRAM ----
            wqk_r = wqk_d[:].rearrange("(kt p) c -> p kt c", p=128)
            with (
                tc.tile_pool(name="wqk", bufs=2) as wpool,
                tc.tile_pool(name="psqk", bufs=2, space="PSUM") as psqk,
            ):
                for m in range(8):
                    wt = wpool.tile([128, NKT, 128], f32r, tag="w")
                    nc.default_dma_engine.dma_start(
                        wt[:], wqk_r[:, :, 128 * m : 128 * (m + 1)]
                    )
                    for nh in range(2):  # halves of T
                        ps = psqk.tile([128, 1024], f32, tag="qk")
                        for kt in range(NKT):
                            for ncx in range(2):
                                nc.tensor.matmul(
                                    ps[:, 512 * ncx : 512 * (ncx + 1)],
                                    (wt[:, kt, :]),
                                    (xT[
                                            :,
                                            kt,
                                            1024 * nh
                                            + 512 * ncx : 1024 * nh
                                            + 512 * (ncx + 1),
                                        ]
                                    ),
                                    start=(kt == 0),
                                    stop=(kt == NKT - 1),
                                )
                        sbt = wpool.tile([128, 1024], bf16, tag="qkout")
                        nc.vector.tensor_scalar_add(sbt[:], ps[:], bqk_sb[:, m : m + 1])
                        nc.default_dma_engine.dma_start(
                            qkT_dts[m // 2][m % 2, :, 1024 * nh : 1024 * (nh + 1)],
                            sbt[:],
                        )

            # ---- Phase 2b: v = x @ W_v (x^T stationary), stays in SBUF ----
            wv_r = wv_d[:].rearrange("(kt p) c -> p kt c", p=128)
            with (
                tc.tile_pool(name="wv", bufs=1) as wvpool,
                tc.tile_pool(name="psv", bufs=2, space="PSUM") as psv,
            ):
                wv_sb = wvpool.tile([128, NKT, 512], f32r)
                nc.default_dma_engine.dma_start(wv_sb[:], wv_r)
                for tt in range(NT):
                    ps = psv.tile([128, 512], f32, tag="v")
                    for kt in range(NKT):
                        nc.tensor.matmul(
                            ps[:],
                            (xT[:, kt, 128 * tt : 128 * (tt + 1)]),
                            (wv_sb[:, kt, :]),
                            start=(kt == 0),
                            stop=(kt == NKT - 1),
                        )
                    nc.vector.tensor_copy(v_all[:, tt, :], ps[:])

            xT_cm.__exit__(None, None, None)

            # ---- Phase 3: attention per head pair ----
            with (
                tc.tile_pool(name="qkp", bufs=2) as qkpool,
                tc.tile_pool(name="pt", bufs=6) as ptpool,
                tc.tile_pool(name="nrm", bufs=4) as nrmpool,
                tc.tile_pool(name="psS", bufs=3, space="PSUM") as psS,
                tc.tile_pool(name="psY", bufs=2, space="PSUM") as psY,
            ):
                for pair in range(NPAIR):
                    qk = qkpool.tile([128, 2, T], bf16, tag="qkp")
                    nc.default_dma_engine.dma_start(
                        qk[:],
                        qkT_dts[pair][:].rearrange("a p t -> p a t"),
                    )
                    for h01 in range(2):
                        h = 2 * pair + h01
                        nc.vector.tensor_copy(
                            vg[:, :, h01, 0:64], v_all[:, :, 64 * h : 64 * (h + 1)]
                        )
                    for j in range(NCH):
                        nk = 4 * (j + 1)  # causal k-tiles for this q-chunk
                        for h01 in range(2):
                            base = 64 * h01
                            psy = psY.tile([128, QCH], f32, tag="y")
                            for g in range(nk // 2):
                                pss = psS.tile([128, 2, QCH], f32, tag="s")
                                for kkk in range(2):
                                    kk = 2 * g + kkk
                                    nc.tensor.matmul(
                                        pss[:, kkk, :],
                                        (qk[
                                                base : base + 64,
                                                1,
                                                128 * kk : 128 * (kk + 1),
                                            ]
                                        ),
                                        (qk[
                                                base : base + 64,
                                                0,
                                                QCH * j : QCH * (j + 1),
                                            ]
                                        ),
                                        start=True,
                                        stop=True,
                                    )
                                pt = ptpool.tile([128, 2, QCH], bf16, tag="pt")
                                nc.scalar.activation(
                                    pt[:],
                                    pss[:],
                                    func=Exp,
                                    scale=0.125,
                                )
                                # causal masking on the diagonal k-tiles
                                for kkk in range(2):
                                    kk = 2 * g + kkk
                                    if kk >= 4 * j:
                                        nc.vector.tensor_mul(
                                            pt[:, kkk, :],
                                            pt[:, kkk, :],
                                            masks_sb[:, kk - 4 * j, :],
                                        )
                                for kkk in range(2):
                                    kk = 2 * g + kkk
                                    # stationary [v_h | ones]: rows 0:64 = y,
                                    # rows 64:128 = softmax sums (replicated)
                                    nc.tensor.matmul(
                                        psy[:],
                                        (vg[:, kk, h01, :]),
                                        (pt[:, kkk, :]),
                                        start=(kk == 0),
                                        stop=(kk == nk - 1),
                                    )
                            yn = nrmpool.tile([128, QCH], f32, tag="yn")
                            nc.vector.tensor_copy(yn[:], psy[:])
                            lns = nrmpool.tile([64, QCH], f32, tag="lns")
                            nc.scalar.activation(
                                lns[:], yn[64:128, :], func=Ln
                            )
                            rc = nrmpool.tile([64, QCH], f32, tag="rc")
                            nc.scalar.activation(
                                rc[:], lns[:], func=Exp, scale=-1.0
                            )
                            nc.vector.tensor_mul(
                                ytn[base : base + 64, pair, QCH * j : QCH * (j + 1)],
                                yn[0:64, :],
                                rc[:],
                            )

            # ---- Phase 4: partial projection out = y^T.T @ W_proj_rows ----
            wp_r = wp_d[:].rearrange("(pr p) c -> p pr c", p=128)
            with (
                tc.tile_pool(name="wp", bufs=1) as wppool,
                tc.tile_pool(name="outp", bufs=3) as outpool,
                tc.tile_pool(name="pspj", bufs=2, space="PSUM") as pspj,
            ):
                wp_sb = wppool.tile([128, NPAIR, D_MODEL], f32r)
                nc.default_dma_engine.dma_start(wp_sb[:], wp_r)
                for tt in range(NT):
                    ob = outpool.tile([128, D_MODEL], f32, tag="ob")
                    for ncx in range(2):
                        ps = pspj.tile([128, 512], f32, tag="pj")
                        for pr in range(NPAIR):
                            nc.tensor.matmul(
                                ps[:],
                                (ytn[:, pr, 128 * tt : 128 * (tt + 1)]),
                                (wp_sb[:, pr, 512 * ncx : 512 * (ncx + 1)]),
                                start=(pr == 0),
                                stop=(pr == NPAIR - 1),
                            )
                        nc.vector.tensor_copy(ob[:, 512 * ncx : 512 * (ncx + 1)], ps[:])
                    nc.default_dma_engine.dma_start(
                        out_d[128 * tt : 128 * (tt + 1), :], ob[:]
                    )

    nc.finalize()
    return nc


def get_nc():
    if "nc" not in _CACHE:
        _CACHE["nc"] = _build_nc()
    return _CACHE["nc"]


def make_host_constants():
    ident = np.eye(128, dtype=np.float32)
    # mask m: k = QCH*j + 128*m + p vs q = QCH*j + c -> valid 128*m + p <= c
    p = np.arange(128)[:, None]
    c = np.arange(QCH)[None, :]
    masks = np.stack(
        [(128 * m + p <= c).astype(np.float32) for m in range(4)]
    )
    return ident, masks


def make_in_maps(x, W_attn, b_attn, W_proj):
    x = np.ascontiguousarray(np.asarray(x, dtype=np.float32))
    W_attn = np.asarray(W_attn, dtype=np.float32)
    b_attn = np.asarray(b_attn, dtype=np.float32)
    W_proj = np.asarray(W_proj, dtype=np.float32)
    import ml_dtypes

    ident, masks = make_host_constants()
    masks_bf = masks.astype(ml_dtypes.bfloat16)
    in_maps = []
    for c in range(N_CORES):
        b, hg = c // 2, c % 2
        h0 = HPC * hg
        # column order per pair: [q_even(64) | q_odd(64)] then [k_even | k_odd]
        qcols, kcols, bcols = [], [], []
        for pr in range(NPAIR):
            he, ho = h0 + 2 * pr, h0 + 2 * pr + 1
            qc = list(range(64 * he, 64 * he + 64)) + list(range(64 * ho, 64 * ho + 64))
            kc = [D_MODEL + i for i in qc]
            qcols.append(qc)
            kcols.append(kc)
        cols = []
        for pr in range(NPAIR):
            cols += qcols[pr] + kcols[pr]
        wqk = np.ascontiguousarray(W_attn[:, cols])
        bqk = np.ascontiguousarray(b_attn[cols].reshape(8, 128).T)
        vcols = list(range(2 * D_MODEL + 64 * h0, 2 * D_MODEL + 64 * (h0 + HPC)))
        wv = np.ascontiguousarray(W_attn[:, vcols])
        wp = np.ascontiguousarray(W_proj[64 * h0 : 64 * (h0 + HPC), :])
        in_maps.append(
            {
                "x": x[b],
                "wqk": wqk,
                "wv": wv,
                "wp": wp,
                "bqk": bqk,
                "ident": ident,
                "masks": masks_bf,
                "ones": np.ones((128, NT, 2, 64), dtype=ml_dtypes.bfloat16),
            }
        )
    return in_maps


def kernel(x, W_attn, b_attn, W_proj, b_proj, **run_kwargs):
    from concourse.bass_utils import run_bass_kernel_spmd

    nc = get_nc()
    in_maps = make_in_maps(x, W_attn, b_attn, W_proj)
    res = run_bass_kernel_spmd(nc, in_maps, list(range(N_CORES)), **run_kwargs)
    _CACHE["last_results"] = res

    b_attn = np.asarray(b_attn, dtype=np.float32)
    W_proj = np.asarray(W_proj, dtype=np.float32)
    b_proj = np.asarray(b_proj, dtype=np.float32)
    bv = b_attn[2 * D_MODEL : 3 * D_MODEL]
    const = (bv @ W_proj + b_proj).astype(np.float32)
    out = np.empty((B, T, D_MODEL), dtype=np.float32)
    for b in range(B):
        out[b] = res.results[2 * b]["out"] + res.results[2 * b + 1]["out"] + const
    return out


# revision 28
# speedup vs baseline: 1.3243x; 1.1815x over previous
"""Causal self-attention (B=4, T=2048, d_model=1024, 16 heads) on 8 NeuronCores.

Sharding: core c = (batch b = c//2, head-group hg = c%2). Each core computes
QKV for its 8 heads, causal attention, and a partial output projection for its
batch. Host sums the two per-batch partials and adds biases (v-bias folds to a
constant through softmax: sum_k P = 1).
"""

import os
import sys

import numpy as np

for _p in ("/opt/trn_rl_repo", os.path.expanduser("~/.axon_site/_ro/trn_rl_repo")):
    if os.path.isdir(_p) and _p not in sys.path:
        sys.path.insert(0, _p)

D_MODEL = 1024
N_HEAD = 16
D_HEAD = 64
B = 4
T = 2048
HPC = 8          # heads per core
N_CORES = 8
NT = T // 128    # 16 T-tiles
NKT = D_MODEL // 128  # 8 contraction tiles
NPAIR = HPC // 2  # 4 head pairs per core
QCH = 512        # q-chunk width in attention
NCH = T // QCH   # 4 q-chunks

_CACHE = {}


def _build_nc():
    import concourse.tile as tile
    from concourse import bacc, mybir

    f32 = mybir.dt.float32
    f32r = mybir.dt.float32r
    bf16 = mybir.dt.bfloat16
    Exp = mybir.ActivationFunctionType.Exp
    Ln = mybir.ActivationFunctionType.Ln

    nc = bacc.Bacc("TRN2")

    x_d = nc.dram_tensor("x", [T, D_MODEL], f32, kind="ExternalInput")
    wqk_d = nc.dram_tensor("wqk", [D_MODEL, 1024], f32r, kind="ExternalInput")
    wv_d = nc.dram_tensor("wv", [D_MODEL, 512], f32r, kind="ExternalInput")
    wp_d = nc.dram_tensor("wp", [512, D_MODEL], f32r, kind="ExternalInput")
    bqk_d = nc.dram_tensor("bqk", [128, 8], f32, kind="ExternalInput")
    ident_d = nc.dram_tensor("ident", [128, 128], f32, kind="ExternalInput")
    masks_d = nc.dram_tensor("masks", [4, 128, QCH], bf16, kind="ExternalInput")
    ones_d = nc.dram_tensor("ones", [128, NT, 2, 64], bf16, kind="ExternalInput")
    out_d = nc.dram_tensor("out", [T, D_MODEL], f32, kind="ExternalOutput")

    with tile.TileContext(nc) as tc:
        with (
            tc.tile_pool(name="persist", bufs=1) as persist,
            tc.tile_pool(name="dram", bufs=1, space="DRAM") as dram,
        ):
            ident_sb = persist.tile([128, 128], f32)
            nc.default_dma_engine.dma_start(ident_sb[:], ident_d[:])
            masks_sb = persist.tile([128, 4, QCH], bf16)
            nc.default_dma_engine.dma_start(
                masks_sb[:], masks_d[:].rearrange("m p c -> p m c")
            )
            bqk_sb = persist.tile([128, 8], f32)
            nc.default_dma_engine.dma_start(bqk_sb[:], bqk_d[:])
            vg = persist.tile([128, NT, 2, 128], bf16)  # [v_h | ones] stationaries
            nc.default_dma_engine.dma_start(vg[:, :, :, 64:128], ones_d[:])

            v_all = persist.tile([128, NT, 512], bf16)   # v natural, [T-tile part, kt, heads*64]
            ytn = persist.tile([128, NPAIR, T], f32r)    # normalized y^T stacked per pair
            qkT_dts = [
                dram.tile([2, 128, T], bf16, tag=f"qkb{p}", name=f"qkb{p}")
                for p in range(NPAIR)
            ]

            xT_cm = tc.tile_pool(name="xTp", bufs=1)
            xT_pool = xT_cm.__enter__()
            xT = xT_pool.tile([128, NKT, T], f32r)       # x transposed

            # ---- Phase 1: load x, transpose 128x128 blocks on TensorE ----
            with (
                tc.tile_pool(name="xin", bufs=6) as xin,
                tc.tile_pool(name="pst", bufs=4, space="PSUM") as pst,
            ):
                for tt in range(NT):
                    xt = xin.tile([128, D_MODEL], f32, tag="xt")
                    nc.default_dma_engine.dma_start(
                        xt[:], x_d[128 * tt : 128 * (tt + 1), :]
                    )
                    for kt in range(NKT):
                        ps = pst.tile([128, 128], f32, tag="ps")
                        nc.tensor.transpose(
                            ps[:], xt[:, 128 * kt : 128 * (kt + 1)], ident_sb[:]
                        )
                        nc.vector.tensor_copy(
                            xT[:, kt, 128 * tt : 128 * (tt + 1)], ps[:]
                        )

            # ---- Phase 2a: q^T/k^T = W_qk^T @ x^T (W stationary), to DRAM ----
            wqk_r = wqk_d[:].rearrange("(kt p) c -> p kt c", p=128)
            with (
                tc.tile_pool(name="wqk", bufs=2) as wpool,
                tc.tile_pool(name="psqk", bufs=2, space="PSUM") as psqk,
            ):
                for m in range(8):
                    wt = wpool.tile([128, NKT, 128], f32r, tag="w")
                    nc.default_dma_engine.dma_start(
                        wt[:], wqk_r[:, :, 128 * m : 128 * (m + 1)]
                    )
                    for nh in range(2):  # halves of T
                        ps = psqk.tile([128, 1024], f32, tag="qk")
                        for kt in range(NKT):
                            for ncx in range(2):
                                nc.tensor.matmul(
                                    ps[:, 512 * ncx : 512 * (ncx + 1)],
                                    (wt[:, kt, :]),
                                    (xT[
                                            :,
                                            kt,
                                            1024 * nh
                                            + 512 * ncx : 1024 * nh
                                            + 512 * (ncx + 1),
                                        ]
                                    ),
                                    start=(kt == 0),
                                    stop=(kt == NKT - 1),
                                )
                        sbt = wpool.tile([128, 1024], bf16, tag="qkout")
                        nc.vector.tensor_scalar_add(sbt[:], ps[:], bqk_sb[:, m : m + 1])
                        nc.default_dma_engine.dma_start(
                            qkT_dts[m // 2][m % 2, :, 1024 * nh : 1024 * (nh + 1)],
                            sbt[:],
                        )

            # ---- Phase 2b: v = x @ W_v (x^T stationary), stays in SBUF ----
            wv_r = wv_d[:].rearrange("(kt p) c -> p kt c", p=128)
            with (
                tc.tile_pool(name="wv", bufs=1) as wvpool,
                tc.tile_pool(name="psv", bufs=2, space="PSUM") as psv,
            ):
                wv_sb = wvpool.tile([128, NKT, 512], f32r)
                nc.default_dma_engine.dma_start(wv_sb[:], wv_r)
                for tt in range(NT):
                    ps = psv.tile([128, 512], f32, tag="v")
                    for kt in range(NKT):
                        nc.tensor.matmul(
                            ps[:],
                            (xT[:, kt, 128 * tt : 128 * (tt + 1)]),
                            (wv_sb[:, kt, :]),
                            start=(kt == 0),
                            stop=(kt == NKT - 1),
                        )
                    nc.vector.tensor_copy(v_all[:, tt, :], ps[:])

            xT_cm.__exit__(None, None, None)

            # ---- Phase 3: attention per head pair ----
            with (
                tc.tile_pool(name="qkp", bufs=2) as qkpool,
                tc.tile_pool(name="pt", bufs=6) as ptpool,
                tc.tile_pool(name="nrm", bufs=4) as nrmpool,
                tc.tile_pool(name="psS", bufs=3, space="PSUM") as psS,
                tc.tile_pool(name="psY", bufs=2, space="PSUM") as psY,
            ):
                for pair in range(NPAIR):
                    yns = []
                    qk = qkpool.tile([128, 2, T], bf16, tag="qkp")
                    nc.default_dma_engine.dma_start(
                        qk[:],
                        qkT_dts[pair][:].rearrange("a p t -> p a t"),
                    )
                    for h01 in range(2):
                        h = 2 * pair + h01
                        nc.vector.tensor_copy(
                            vg[:, :, h01, 0:64], v_all[:, :, 64 * h : 64 * (h + 1)]
                        )
                    for j in range(NCH):
                        nk = 4 * (j + 1)  # causal k-tiles for this q-chunk
                        for h01 in range(2):
                            base = 64 * h01
                            psy = psY.tile([128, QCH], f32, tag="y")
                            for g in range(nk // 2):
                                pss = psS.tile([128, 2, QCH], f32, tag="s")
                                for kkk in range(2):
                                    kk = 2 * g + kkk
                                    nc.tensor.matmul(
                                        pss[:, kkk, :],
                                        (qk[
                                                base : base + 64,
                                                1,
                                                128 * kk : 128 * (kk + 1),
                                            ]
                                        ),
                                        (qk[
                                                base : base + 64,
                                                0,
                                                QCH * j : QCH * (j + 1),
                                            ]
                                        ),
                                        start=True,
                                        stop=True,
                                    )
                                pt = ptpool.tile([128, 2, QCH], bf16, tag="pt")
                                nc.scalar.activation(
                                    pt[:],
                                    pss[:],
                                    func=Exp,
                                    scale=0.125,
                                )
                                # causal masking on the diagonal k-tiles
                                for kkk in range(2):
                                    kk = 2 * g + kkk
                                    if kk >= 4 * j:
                                        nc.vector.tensor_mul(
                                            pt[:, kkk, :],
                                            pt[:, kkk, :],
                                            masks_sb[:, kk - 4 * j, :],
                                        )
                                for kkk in range(2):
                                    kk = 2 * g + kkk
                                    # stationary [v_h | ones]: rows 0:64 = y,
                                    # rows 64:128 = softmax sums (replicated)
                                    nc.tensor.matmul(
                                        psy[:],
                                        (vg[:, kk, h01, :]),
                                        (pt[:, kkk, :]),
                                        start=(kk == 0),
                                        stop=(kk == nk - 1),
                                    )
                            yn = nrmpool.tile([128, QCH], f32, tag="yn", bufs=8)
                            nc.vector.tensor_copy(yn[:], psy[:])
                            yns.append((yn, h01, j))

                    # batched softmax normalization: all Ln's then all Exp's
                    # (one activation-table swap per op type per pair)
                    lns_l = []
                    for yn, h01, j in yns:
                        lns = nrmpool.tile([64, QCH], f32, tag="lns", bufs=8)
                        nc.scalar.activation(lns[:], yn[64:128, :], func=Ln)
                        lns_l.append(lns)
                    for (yn, h01, j), lns in zip(yns, lns_l):
                        rc = nrmpool.tile([64, QCH], f32, tag="rc", bufs=8)
                        nc.scalar.activation(rc[:], lns[:], func=Exp, scale=-1.0)
                        nc.vector.tensor_mul(
                            ytn[
                                64 * h01 : 64 * h01 + 64,
                                pair,
                                QCH * j : QCH * (j + 1),
                            ],
                            yn[0:64, :],
                            rc[:],
                        )

            # ---- Phase 4: partial projection out = y^T.T @ W_proj_rows ----
            wp_r = wp_d[:].rearrange("(pr p) c -> p pr c", p=128)
            with (
                tc.tile_pool(name="wp", bufs=1) as wppool,
                tc.tile_pool(name="outp", bufs=3) as outpool,
                tc.tile_pool(name="pspj", bufs=2, space="PSUM") as pspj,
            ):
                wp_sb = wppool.tile([128, NPAIR, D_MODEL], f32r)
                nc.default_dma_engine.dma_start(wp_sb[:], wp_r)
                for tt in range(NT):
                    ob = outpool.tile([128, D_MODEL], f32, tag="ob")
                    for ncx in range(2):
                        ps = pspj.tile([128, 512], f32, tag="pj")
                        for pr in range(NPAIR):
                            nc.tensor.matmul(
                                ps[:],
                                (ytn[:, pr, 128 * tt : 128 * (tt + 1)]),
                                (wp_sb[:, pr, 512 * ncx : 512 * (ncx + 1)]),
                                start=(pr == 0),
                                stop=(pr == NPAIR - 1),
                            )
                        nc.vector.tensor_copy(ob[:, 512 * ncx : 512 * (ncx + 1)], ps[:])
                    nc.default_dma_engine.dma_start(
                        out_d[128 * tt : 128 * (tt + 1), :], ob[:]
                    )

    nc.finalize()
    return nc


def get_nc():
    if "nc" not in _CACHE:
        _CACHE["nc"] = _build_nc()
    return _CACHE["nc"]


def make_host_constants():
    ident = np.eye(128, dtype=np.float32)
    # mask m: k = QCH*j + 128*m + p vs q = QCH*j + c -> valid 128*m + p <= c
    p = np.arange(128)[:, None]
    c = np.arange(QCH)[None, :]
    masks = np.stack(
        [(128 * m + p <= c).astype(np.float32) for m in range(4)]
    )
    return ident, masks


def make_in_maps(x, W_attn, b_attn, W_proj):
    x = np.ascontiguousarray(np.asarray(x, dtype=np.float32))
    W_attn = np.asarray(W_attn, dtype=np.float32)
    b_attn = np.asarray(b_attn, dtype=np.float32)
    W_proj = np.asarray(W_proj, dtype=np.float32)
    import ml_dtypes

    ident, masks = make_host_constants()
    masks_bf = masks.astype(ml_dtypes.bfloat16)
    in_maps = []
    for c in range(N_CORES):
        b, hg = c // 2, c % 2
        h0 = HPC * hg
        # column order per pair: [q_even(64) | q_odd(64)] then [k_even | k_odd]
        qcols, kcols, bcols = [], [], []
        for pr in range(NPAIR):
            he, ho = h0 + 2 * pr, h0 + 2 * pr + 1
            qc = list(range(64 * he, 64 * he + 64)) + list(range(64 * ho, 64 * ho + 64))
            kc = [D_MODEL + i for i in qc]
            qcols.append(qc)
            kcols.append(kc)
        cols = []
        for pr in range(NPAIR):
            cols += qcols[pr] + kcols[pr]
        wqk = np.ascontiguousarray(W_attn[:, cols])
        bqk = np.ascontiguousarray(b_attn[cols].reshape(8, 128).T)
        vcols = list(range(2 * D_MODEL + 64 * h0, 2 * D_MODEL + 64 * (h0 + HPC)))
        wv = np.ascontiguousarray(W_attn[:, vcols])
        wp = np.ascontiguousarray(W_proj[64 * h0 : 64 * (h0 + HPC), :])
        in_maps.append(
            {
                "x": x[b],
                "wqk": wqk,
                "wv": wv,
                "wp": wp,
                "bqk": bqk,
                "ident": ident,
                "masks": masks_bf,
                "ones": np.ones((128, NT, 2, 64), dtype=ml_dtypes.bfloat16),
            }
        )
    return in_maps


def kernel(x, W_attn, b_attn, W_proj, b_proj, **run_kwargs):
    from concourse.bass_utils import run_bass_kernel_spmd

    nc = get_nc()
    in_maps = make_in_maps(x, W_attn, b_attn, W_proj)
    res = run_bass_kernel_spmd(nc, in_maps, list(range(N_CORES)), **run_kwargs)
    _CACHE["last_results"] = res

    b_attn = np.asarray(b_attn, dtype=np.float32)
    W_proj = np.asarray(W_proj, dtype=np.float32)
    b_proj = np.asarray(b_proj, dtype=np.float32)
    bv = b_attn[2 * D_MODEL : 3 * D_MODEL]
    const = (bv @ W_proj + b_proj).astype(np.float32)
    out = np.empty((B, T, D_MODEL), dtype=np.float32)
    for b in range(B):
        out[b] = res.results[2 * b]["out"] + res.results[2 * b + 1]["out"] + const
    return out
